# revision 50
# baseline (speedup 1.0000x reference)
"""MultiHeadAttention (GQA + symmetric ALiBi) on 8 trn2 NeuronCores.

Sharding: core = (batch n in {0,1}) x (head-group g in {0..3}).
Each core handles 4 query heads (one GQA pair of kv heads) for one batch.
All matmuls run in bf16 with fp32 PSUM accumulation. Stationaries are
zero-padded to 128x128 so the fast weight-load path stays enabled.

The kernel is a single fused pipeline paced by the ACT engine's exp stream
(the provable floor: 128 exp tiles of [128,1024] ~ 1.15us each):
  - lead-in: K, Q(h0,h1) projections (x staged once in SBUF), then the S
    stream for head 0 starts immediately; V projection + PE transposes and
    Q(h2,h3) projections are woven into the stream afterwards.
  - per (head, kc): S^T = K^T.T @ Q^T (PSUM, tag "s" double-buffered), ACT
    exp(S*0.125) -> bf16, DVE multiply by a host-built ALiBi exp table
    (texp, indexed by k - q + 1920), giving P tiles in SBUF.
  - PV matmuls (stationary = V with a ones column for row sums) are emitted
    through a deferred queue a few kc behind the S stream, so PSUM-slot
    waits never stall the S/exp pipeline.
  - normalization: sums DMA'd [1,2048]->[128,16], DVE reciprocal, DMA back,
    gpsimd partition_broadcast, DVE multiply straight out of PV PSUM. No PE
    or PSUM-slot involvement.
  - output projection: per ec, [128,1024] PSUM tiles; bias fused into the
    PSUM->SBUF copy, alternating between ACT (activation+bias) and DVE
    (tensor_scalar_add) so neither engine gates the drain.
bf16 partials per head-group; host sums the 4 group partials per batch.
"""
import sys

sys.path.insert(0, "/opt/trn_rl_repo")
import numpy as np
import ml_dtypes

import concourse.bass as bass
import concourse.mybir as mybir
from concourse import bacc
from concourse.tile import TileContext
from concourse.masks import make_identity
from concourse.bass_utils import run_bass_kernel_spmd


def _register_ntff_hook_module():
    # bass_utils imports antenv.axon_hooks for trace=True under axon; this
    # image's antenv lacks it, so register a shim in sys.modules and set the
    # hook the same way trn_boot would have.
    import types

    if "antenv.axon_hooks" in sys.modules:
        return
    try:
        mod = types.ModuleType("antenv.axon_hooks")
        _hook = [None]
        mod.set_axon_ntff_profile_hook = lambda h: _hook.__setitem__(0, h)
        mod.get_axon_ntff_profile_hook = lambda: _hook[0]
        sys.modules["antenv.axon_hooks"] = mod
        from trn_agent_boot.trn_boot import _ntff_profile_via_ctypes

        mod.set_axon_ntff_profile_hook(
            _ntff_profile_via_ctypes("/opt/axon/libaxon_pjrt.so")
        )
    except Exception:
        pass


_register_ntff_hook_module()

S = 2048
E = 1024
D = 64
TW = 3968  # alibi exp-table width: u = j - k0 + 1920 in [0, 3968)
F32 = mybir.dt.float32
BF16 = mybir.dt.bfloat16

_NC = None
LAST_RESULTS = None


def _build():
    nc = bacc.Bacc("TRN2", target_bir_lowering=False, debug=False, num_devices=8)
    xT = nc.dram_tensor("xT", [E, S], BF16, kind="ExternalInput")
    wqT = nc.dram_tensor("wqT", [E, 256], BF16, kind="ExternalInput")
    wkT = nc.dram_tensor("wkT", [E, 128], BF16, kind="ExternalInput")
    wvT = nc.dram_tensor("wvT", [E, 128], BF16, kind="ExternalInput")
    woT = nc.dram_tensor("woT", [256, E], BF16, kind="ExternalInput")
    bo4 = nc.dram_tensor("bo4", [128, 8], F32, kind="ExternalInput")
    texp = nc.dram_tensor("texp", [4, 128, TW], BF16, kind="ExternalInput")
    outT = nc.dram_tensor("outT", [E, S], BF16, kind="ExternalOutput")

    Exp = mybir.ActivationFunctionType.Exp
    Ident = mybir.ActivationFunctionType.Identity

    with TileContext(nc) as tc:
        with (
            tc.sbuf_pool(name="const", bufs=1) as const,
            tc.sbuf_pool(name="pp", bufs=1) as pp,
            tc.sbuf_pool(name="nrm", bufs=2) as nrm,
            tc.sbuf_pool(name="osb", bufs=4) as osb,
            tc.psum_pool(name="ps", bufs=1) as psp,
        ):
            # ---- persistent SBUF
            x_sb = const.tile([128, 16 * 1024], BF16)  # chunk c = qh*8 + e
            wq_sb = const.tile([128, 8 * 256], BF16)
            wk_sb = const.tile([128, 8 * 128], BF16)
            wv_sb = const.tile([128, 8 * 128], BF16)
            wo_sb = const.tile([128, 2 * 1024], BF16)
            bo_sb = const.tile([128, 8], F32)
            tex_sb = const.tile([128, 4 * TW], BF16)
            ident = const.tile([128, 128], BF16)
            QT = [const.tile([128, S], BF16, name=f"qt{h}") for h in range(4)]
            KT = [const.tile([128, S], BF16, name=f"kt{k}") for k in range(2)]
            vt_sb = const.tile([128, S], BF16)
            VS = [const.tile([128, 16 * 128], BF16, name=f"vs{k}") for k in range(2)]
            AT = [const.tile([128, S], BF16, name=f"at{c}") for c in range(2)]

            # ---- DMAs, priority order (sync queue is FIFO)
            nc.sync.dma_start(
                out=wk_sb.rearrange("p (c m) -> p c m", m=128),
                in_=wkT.rearrange("(c p) m -> p c m", p=128),
            )
            # x in 8 chunks of 2 e-blocks so the first projection matmuls
            # start ~2us in, with wq woven early for the Q blocks.
            for gi in range(8):
                qh, e0 = gi // 4, (gi % 4) * 2
                nc.sync.dma_start(
                    out=x_sb[
                        :, (qh * 8 + e0) * 1024 : (qh * 8 + e0 + 2) * 1024
                    ].rearrange("p (c m) -> p c m", m=1024),
                    in_=xT[e0 * 128 : (e0 + 2) * 128, qh * 1024 : (qh + 1) * 1024]
                    .rearrange("(c p) m -> p c m", p=128),
                )
                if gi == 1:
                    nc.sync.dma_start(
                        out=wq_sb.rearrange("p (c m) -> p c m", m=256),
                        in_=wqT.rearrange("(c p) m -> p c m", p=128),
                    )
            nc.sync.dma_start(
                out=wv_sb.rearrange("p (c m) -> p c m", m=128),
                in_=wvT.rearrange("(c p) m -> p c m", p=128),
            )
            nc.sync.dma_start(out=tex_sb[:, 0:TW], in_=texp[0])
            nc.sync.dma_start(out=bo_sb, in_=bo4[:, :])
            nc.sync.dma_start(
                out=wo_sb.rearrange("p (c m) -> p c m", m=1024),
                in_=woT.rearrange("(c p) m -> p c m", p=128),
            )
            for t in range(1, 4):
                nc.sync.dma_start(
                    out=tex_sb[:, t * TW : (t + 1) * TW], in_=texp[t]
                )

            # ---- one-time zeroing (S/PV stationaries + padded contraction
            # rows).  First-needed ones on DVE, the rest on idle gpsimd.
            nc.vector.memset(QT[0][64:128, :], 0.0)
            nc.vector.memset(KT[0][64:128, :], 0.0)
            nc.gpsimd.memset(KT[1][64:128, :], 0.0)
            for h in range(1, 4):
                nc.gpsimd.memset(QT[h][64:128, :], 0.0)
            for kv in range(2):
                nc.gpsimd.memset(VS[kv], 0.0)
                nc.gpsimd.memset(
                    VS[kv].rearrange("p (c m) -> p c m", m=128)[:, :, 64:65], 1.0
                )
            make_identity(nc, ident)

            def xc(qh, e):
                c = qh * 8 + e
                return x_sb[:, c * 1024 : (c + 1) * 1024]

            # ---- lead-in projections: 3-slot rotation (2x "s" + the idle
            # "pv" slot) so block i+2 never waits on block i's copies.
            def proj(wof, qh, copies, nm, tag="s"):
                bufs = 2 if tag == "s" else 1
                pst = psp.tile([128, 1024], F32, tag=tag, bufs=bufs, name=nm)
                for e in range(8):
                    x_ = xc(qh, e)
                    w = wof(e)
                    for i in range(2):
                        nc.tensor.matmul(
                            pst[:, i * 512 : (i + 1) * 512],
                            w,
                            x_[:, i * 512 : (i + 1) * 512],
                            start=(e == 0),
                            stop=(e == 7),
                        )
                copies(pst, qh)

            def k_copies(pst, qh):
                qs = slice(qh * 1024, (qh + 1) * 1024)
                nc.scalar.copy(KT[0][0:64, qs], pst[0:64, :])
                nc.scalar.copy(KT[1][0:64, qs], pst[64:128, :])

            def q01_copies(pst, qh):
                qs = slice(qh * 1024, (qh + 1) * 1024)
                nc.scalar.copy(QT[0][0:64, qs], pst[0:64, :])
                nc.scalar.copy(QT[1][0:64, qs], pst[64:128, :])

            def v_copies(pst, qh):
                qs = slice(qh * 1024, (qh + 1) * 1024)
                nc.vector.tensor_copy(vt_sb[:, qs], pst)

            # only K and Q(h0,h1) gate the S stream; V is projected inside
            # head 0 through the pv slot.
            wk_of = lambda e: wk_sb[:, e * 128 : (e + 1) * 128]
            wq_of = lambda e: wq_sb[:, e * 256 : e * 256 + 128]
            wv_of = lambda e: wv_sb[:, e * 128 : (e + 1) * 128]
            proj(wk_of, 0, k_copies, "pk", "s")
            proj(wq_of, 0, q01_copies, "pq", "s")
            proj(wq_of, 1, q01_copies, "pq", "pv")
            proj(wk_of, 1, k_copies, "pk", "s")

            # ---- weavable blocks (all live in the tag-"pv" slot, which is
            # idle until the first PV, so the S/exp stream never blocks)
            def big_transpose():
                # all 16 V^T->V transposes into ONE psum tile, then two big
                # strided copies into the VS stationaries (instead of 16
                # slot-serialized round-trips).
                bigpt = psp.tile([128, 2048], BF16, tag="pv", bufs=1, name="bigpt")
                for i in range(16):
                    nc.tensor.transpose(
                        bigpt[:, i * 128 : (i + 1) * 128],
                        vt_sb[:, i * 128 : (i + 1) * 128],
                        ident,
                    )
                for kv in range(2):
                    nc.vector.tensor_copy(
                        VS[kv].rearrange("p (c m) -> p c m", m=128)[:, :, 0:64],
                        bigpt.rearrange("p (c m) -> p c m", m=128)[
                            :, :, kv * 64 : (kv + 1) * 64
                        ],
                    )

            # half-block chunks through the pv slot, so each kc gets at most
            # ~8 extra matmuls and the ACT stream never starves.
            chunk_hold = {}

            def proj_chunk(key, wof, qh, part, copies):
                # half-block (8 matmul) chunks through the pv slot
                if part == 0:
                    chunk_hold[key] = psp.tile(
                        [128, 1024], F32, tag="pv", bufs=1, name=key
                    )
                pst = chunk_hold[key]
                for e in range(part * 4, part * 4 + 4):
                    x_ = xc(qh, e)
                    w = wof(e)
                    for i in range(2):
                        nc.tensor.matmul(
                            pst[:, i * 512 : (i + 1) * 512],
                            w,
                            x_[:, i * 512 : (i + 1) * 512],
                            start=(e == 0),
                            stop=(e == 7),
                        )
                if part == 1:
                    copies(pst, qh)

            wq23_of = lambda e: wq_sb[:, e * 256 + 128 : e * 256 + 256]

            def q23_copies(pst, qh):
                qs = slice(qh * 1024, (qh + 1) * 1024)
                nc.vector.tensor_copy(QT[2][0:64, qs], pst[0:64, :])
                nc.vector.tensor_copy(QT[3][0:64, qs], pst[64:128, :])

            # ---- attention stream
            def s_exp_mul(h, kc):
                kv = h // 2
                ptiles = []
                for q2 in range(2):
                    ss = psp.tile([128, 1024], F32, tag="s", bufs=2, name="ss")
                    for i in range(2):
                        qq = q2 * 2 + i
                        nc.tensor.matmul(
                            ss[:, i * 512 : (i + 1) * 512],
                            KT[kv][:, kc * 128 : (kc + 1) * 128],
                            QT[h][:, qq * 512 : (qq + 1) * 512],
                            start=True,
                            stop=True,
                        )
                    pexp = pp.tile([128, 1024], BF16, tag="pexp", bufs=3, name="pexp")
                    nc.scalar.activation(pexp, ss, Exp, scale=0.125)
                    ptile = pp.tile([128, 1024], BF16, tag="p", bufs=18, name="p")
                    u0 = h * TW + 1920 - kc * 128 + q2 * 1024
                    nc.vector.tensor_mul(ptile, pexp, tex_sb[:, u0 : u0 + 1024])
                    ptiles.append(ptile)
                return ptiles

            pvq = []
            pv_tiles = {}

            def norm_start(h):
                # Chain runs entirely off PE/PSUM-slots: DMA reshape, DVE
                # reciprocal, DMA back, gpsimd broadcast, DVE scale from PSUM.
                # Processed in pipelined q-halves to halve the latency until
                # the pv PSUM tile is released (it gates the next head's PV).
                # pv[0:64] is copied to SBUF immediately so the pv PSUM slot
                # frees ~4us after the last PV instead of after the whole
                # chain; the at-mul reads the SBUF copy.  This lets the next
                # head's first PV pop a few kc in with no catch-up lumps.
                pvt = pv_tiles[h]
                at = AT[h // 2]
                r0 = 64 * (h % 2)
                cs = [slice(half * 1024, (half + 1) * 1024) for half in range(2)]
                rsums, s128s, rrbs, rs, rbss, pvss = [], [], [], [], [], []
                # both sums copies first: they gate the DMA->recip->broadcast
                # chains, while the numerator copies only gate the at-muls.
                for half in range(2):
                    rsum = nrm.tile([1, 1024], F32, tag="rsum", name="rsum")
                    nc.vector.tensor_copy(rsum, pvt[64:65, cs[half]])
                    rsums.append(rsum)
                for half in range(2):
                    pvs = nrm.tile([64, 1024], BF16, tag="pvs", name="pvs")
                    nc.vector.tensor_copy(pvs, pvt[0:64, cs[half]])
                    pvss.append(pvs)
                for half in range(2):
                    s128 = nrm.tile([128, 8], F32, tag="s128", name="s128")
                    nc.sync.dma_start(out=s128, in_=rsums[half])
                    s128s.append(s128)
                for half in range(2):
                    rrb = nrm.tile([128, 8], BF16, tag="rrb", name="rrb")
                    with nc.allow_low_precision("1/rowsum rounds to bf16 anyway"):
                        nc.vector.reciprocal(rrb, s128s[half])
                    rrbs.append(rrb)
                for half in range(2):
                    r = nrm.tile([1, 1024], BF16, tag="r", name="r")
                    nc.sync.dma_start(out=r, in_=rrbs[half])
                    rs.append(r)
                for half in range(2):
                    rbs = nrm.tile([64, 1024], BF16, tag="rbs", name="rbs")
                    nc.gpsimd.partition_broadcast(rbs, rs[half])
                    rbss.append(rbs)
                # at-muls are pure-SBUF and latency-slack for h<3 (AT is only
                # read by phase D), so they run on the idle gpsimd engine;
                # h3's stay on DVE since they gate D's c1 matmuls.
                for half in range(2):
                    eng = nc.vector if h == 3 else nc.gpsimd
                    eng.tensor_mul(
                        at[r0 : r0 + 64, cs[half]], pvss[half], rbss[half]
                    )

            def pump(n):
                for _ in range(n):
                    if not pvq:
                        return
                    h, kc, pt_ = pvq.pop(0)
                    if h not in pv_tiles:
                        pv_tiles[h] = psp.tile(
                            [128, 2048], F32, tag="pv", bufs=1, name=f"pv{h}"
                        )
                    pvt = pv_tiles[h]
                    kv = h // 2
                    for qq in range(4):
                        nc.tensor.matmul(
                            pvt[:, qq * 512 : (qq + 1) * 512],
                            VS[kv][:, kc * 128 : (kc + 1) * 128],
                            pt_[qq // 2][:, (qq % 2) * 512 : (qq % 2 + 1) * 512],
                            start=(kc == 0),
                            stop=(kc == 15),
                            skip_group_check=True,
                        )
                    if kc == 15:
                        norm_start(h)

            # Schedule: each head's PVs drain within the head (lag ~6 kc,
            # gentle catch-up at kc12-15), so the normalization chain fires
            # right at head end and the next head's first PV (which waits on
            # it via the pv slot) only pops ~14us later.  The pv-tag slot is
            # FIFO: big_transpose and the q23 blocks must all precede the
            # first pump (which allocates pv(h0)).
            for h in range(4):
                for kc in range(16):
                    s_exp_mul_kc = s_exp_mul(h, kc)
                    pvq.append((h, kc, s_exp_mul_kc))
                    if h == 0:
                        # V projection 8mm/kc over kc0-3, transposes at kc5
                        if kc < 4:
                            proj_chunk("v0" if kc < 2 else "v1", wv_of, kc // 2,
                                       kc % 2, v_copies)
                        elif kc == 5:
                            big_transpose()
                        if 7 <= kc <= 11:
                            pump(1)
                        elif 12 <= kc <= 13:
                            pump(2)
                        elif kc >= 14:
                            pump(3 if kc == 14 else 4)
                    elif h == 1:
                        # Q(h2,h3) projections 8mm/kc at kc2-5 (pv0 frees
                        # right at the boundary now, so kc2 is safe).
                        if 2 <= kc <= 5:
                            proj_chunk("q23a" if kc < 4 else "q23b", wq23_of,
                                       (kc - 2) // 2, kc % 2, q23_copies)
                        if 6 <= kc <= 11:
                            pump(1)
                        elif 12 <= kc <= 15:
                            pump(2 if kc < 14 else 3)
                    else:
                        if 4 <= kc <= 11:
                            pump(1)
                        elif 12 <= kc <= 15:
                            pump(2)
            pump(len(pvq))

            # ---- output projection (+ bias/4 fused into the PSUM drain)
            # Even ec use the (now free) 4-bank "pv" slot, odd ec the two
            # "s" slots, so three tiles pipeline through PSUM.
            def d_drain(os_, ec, half):
                o_sb = osb.tile([128, 1024], BF16, tag="osb", name="o_sb")
                nc.vector.tensor_scalar_add(o_sb, os_, bo_sb[:, ec : ec + 1])
                nc.sync.dma_start(
                    out=outT[
                        ec * 128 : (ec + 1) * 128,
                        half * 1024 : (half + 1) * 1024,
                    ],
                    in_=o_sb,
                )

            # half 0 first: its c1 matmuls only need the h3 at-mul for
            # columns 0:1023, which lands ~2us before the half-1 one.  ec6/7
            # ride the freed pv slot between the passes for a 3rd buffer.
            # 4 ecs drain through the pv slot as (ec, full 2048): one wide
            # activation + one contiguous DMA each; the other 4 ecs go as
            # (ec, half) tiles through the two "s" slots.  All c0 matmuls of
            # a group precede its c1s so they overlap the h3 norm chain.
            def d_group(ec_s, ec_w, half_tiles_first):
                tiles = []
                for ec, half in half_tiles_first:
                    os_ = psp.tile([128, 1024], F32, tag="s", bufs=2, name="os")
                    tiles.append((os_, 0, [(ec, 0, half * 2), (ec, 512, half * 2 + 1)]))
                os2 = psp.tile([128, 2048], F32, tag="pv", bufs=1, name="os2")
                tiles.append(
                    (os2, 1, [(ec_w, qq * 512, qq) for qq in range(4)])
                )
                for c in range(2):
                    for os_, _, mms in tiles:
                        for ec, off, qq in mms:
                            w = wo_sb[
                                :, c * 1024 + ec * 128 : c * 1024 + (ec + 1) * 128
                            ]
                            nc.tensor.matmul(
                                os_[:, off : off + 512],
                                w,
                                AT[c][:, qq * 512 : (qq + 1) * 512],
                                start=(c == 0),
                                stop=(c == 1),
                                skip_group_check=True,
                            )
                for os_, wide, mms in tiles:
                    if wide:
                        ec = mms[0][0]
                        o_sb = osb.tile(
                            [128, 2048], BF16, tag="osbw", bufs=2, name="o_sb"
                        )
                        nc.scalar.activation(
                            o_sb, os_, Ident, bias=bo_sb[:, ec : ec + 1]
                        )
                        nc.sync.dma_start(
                            out=outT[ec * 128 : (ec + 1) * 128, :], in_=o_sb
                        )
                    else:
                        d_drain(os_, mms[0][0], mms[0][2] // 2)

            # ec 1,3,5,7 via wide pv tiles; ec 0,2,4,6 via per-half s tiles
            d_group(None, 1, [(0, 0), (2, 0)])
            d_group(None, 3, [(4, 0), (6, 0)])
            d_group(None, 5, [(0, 1), (2, 1)])
            d_group(None, 7, [(4, 1), (6, 1)])

    nc.compile()
    return nc


def _texp_tables():
    i = np.arange(128, dtype=np.float64).reshape(128, 1)
    u = np.arange(TW, dtype=np.float64).reshape(1, TW)
    dist = np.abs(i + 1920.0 - u)
    tabs = []
    for g in range(4):
        tg = np.empty([4, 128, TW], dtype=ml_dtypes.bfloat16)
        for hh in range(4):
            slope = 2.0 ** (-(4 * g + hh + 1))
            tg[hh] = np.exp(-slope * dist / 8.0).astype(ml_dtypes.bfloat16)
        tabs.append(tg)
    return tabs


def kernel(x, Wq, Wk, Wv, Wo, bo, _trace=False, _trace_kwargs=None):
    global _NC, LAST_RESULTS
    x = np.asarray(x, dtype=np.float32)
    Wq = np.asarray(Wq, dtype=np.float32)
    Wk = np.asarray(Wk, dtype=np.float32)
    Wv = np.asarray(Wv, dtype=np.float32)
    Wo = np.asarray(Wo, dtype=np.float32)
    bo = np.asarray(bo, dtype=np.float32)

    if _NC is None:
        _NC = _build()
    nc = _NC

    tabs = _texp_tables()
    bf = ml_dtypes.bfloat16
    bo4 = np.ascontiguousarray((bo * 0.25).reshape(8, 128).T).astype(np.float32)
    in_maps = []
    for core in range(8):
        n, g = core // 4, core % 4
        hs = slice(4 * g * D, (4 * g + 4) * D)
        kvs = slice(2 * g * D, (2 * g + 2) * D)
        in_maps.append(
            {
                "xT": np.ascontiguousarray(x[n].T).astype(bf),
                "wqT": np.ascontiguousarray(Wq[hs].T).astype(bf),
                "wkT": np.ascontiguousarray(Wk[kvs].T).astype(bf),
                "wvT": np.ascontiguousarray(Wv[kvs].T).astype(bf),
                "woT": np.ascontiguousarray(Wo[:, hs].T).astype(bf),
                "bo4": bo4,
                "texp": tabs[g],
            }
        )

    kw = {}
    if _trace:
        kw["trace"] = True
        kw.update(_trace_kwargs or {})
    res = run_bass_kernel_spmd(nc, in_maps, list(range(8)), **kw)
    LAST_RESULTS = res

    out = np.empty((2, S, E), dtype=np.float32)
    for n in range(2):
        acc = res.results[n * 4]["outT"].astype(np.float32)
        for g in range(1, 4):
            acc = acc + res.results[n * 4 + g]["outT"]
        out[n] = acc.T
    return out


# revision 51
# speedup vs baseline: 1.0088x; 1.0088x over previous
"""MultiHeadAttention (GQA + symmetric ALiBi) on 8 trn2 NeuronCores.

Sharding: core = (batch n in {0,1}) x (head-group g in {0..3}).
Each core handles 4 query heads (one GQA pair of kv heads) for one batch.
All matmuls run in bf16 with fp32 PSUM accumulation. Stationaries are
zero-padded to 128x128 so the fast weight-load path stays enabled.

The kernel is a single fused pipeline paced by the ACT engine's exp stream
(the provable floor: 128 exp tiles of [128,1024] ~ 1.15us each):
  - lead-in: K, Q(h0,h1) projections (x staged once in SBUF), then the S
    stream for head 0 starts immediately; V projection + PE transposes and
    Q(h2,h3) projections are woven into the stream afterwards.
  - per (head, kc): S^T = K^T.T @ Q^T (PSUM, tag "s" double-buffered), ACT
    exp(S*0.125) -> bf16, DVE multiply by a host-built ALiBi exp table
    (texp, indexed by k - q + 1920), giving P tiles in SBUF.
  - PV matmuls (stationary = V with a ones column for row sums) are emitted
    through a deferred queue a few kc behind the S stream, so PSUM-slot
    waits never stall the S/exp pipeline.
  - normalization: sums DMA'd [1,2048]->[128,16], DVE reciprocal, DMA back,
    gpsimd partition_broadcast, DVE multiply straight out of PV PSUM. No PE
    or PSUM-slot involvement.
  - output projection: per ec, [128,1024] PSUM tiles; bias fused into the
    PSUM->SBUF copy, alternating between ACT (activation+bias) and DVE
    (tensor_scalar_add) so neither engine gates the drain.
bf16 partials per head-group; host sums the 4 group partials per batch.
"""
import sys

sys.path.insert(0, "/opt/trn_rl_repo")
import numpy as np
import ml_dtypes

import concourse.bass as bass
import concourse.mybir as mybir
from concourse import bacc
from concourse.tile import TileContext
from concourse.masks import make_identity
from concourse.bass_utils import run_bass_kernel_spmd


def _register_ntff_hook_module():
    # bass_utils imports antenv.axon_hooks for trace=True under axon; this
    # image's antenv lacks it, so register a shim in sys.modules and set the
    # hook the same way trn_boot would have.
    import types

    if "antenv.axon_hooks" in sys.modules:
        return
    try:
        mod = types.ModuleType("antenv.axon_hooks")
        _hook = [None]
        mod.set_axon_ntff_profile_hook = lambda h: _hook.__setitem__(0, h)
        mod.get_axon_ntff_profile_hook = lambda: _hook[0]
        sys.modules["antenv.axon_hooks"] = mod
        from trn_agent_boot.trn_boot import _ntff_profile_via_ctypes

        mod.set_axon_ntff_profile_hook(
            _ntff_profile_via_ctypes("/opt/axon/libaxon_pjrt.so")
        )
    except Exception:
        pass


_register_ntff_hook_module()

S = 2048
E = 1024
D = 64
TW = 3968  # alibi exp-table width: u = j - k0 + 1920 in [0, 3968)
F32 = mybir.dt.float32
BF16 = mybir.dt.bfloat16

_NC = None
LAST_RESULTS = None


def _build():
    nc = bacc.Bacc("TRN2", target_bir_lowering=False, debug=False, num_devices=8)
    xT = nc.dram_tensor("xT", [E, S], BF16, kind="ExternalInput")
    wqT = nc.dram_tensor("wqT", [E, 256], BF16, kind="ExternalInput")
    wkT = nc.dram_tensor("wkT", [E, 128], BF16, kind="ExternalInput")
    wvT = nc.dram_tensor("wvT", [E, 128], BF16, kind="ExternalInput")
    woT = nc.dram_tensor("woT", [256, E], BF16, kind="ExternalInput")
    bo4 = nc.dram_tensor("bo4", [128, 8], F32, kind="ExternalInput")
    texp = nc.dram_tensor("texp", [4, 128, TW], BF16, kind="ExternalInput")
    outT = nc.dram_tensor("outT", [E, S], BF16, kind="ExternalOutput")

    Exp = mybir.ActivationFunctionType.Exp
    Ident = mybir.ActivationFunctionType.Identity

    with TileContext(nc) as tc:
        with (
            tc.sbuf_pool(name="const", bufs=1) as const,
            tc.sbuf_pool(name="pp", bufs=1) as pp,
            tc.sbuf_pool(name="nrm", bufs=2) as nrm,
            tc.sbuf_pool(name="osb", bufs=4) as osb,
            tc.psum_pool(name="ps", bufs=1) as psp,
        ):
            # ---- persistent SBUF
            x_sb = const.tile([128, 16 * 1024], BF16)  # chunk c = qh*8 + e
            wq_sb = const.tile([128, 8 * 256], BF16)
            wk_sb = const.tile([128, 8 * 128], BF16)
            wv_sb = const.tile([128, 8 * 128], BF16)
            wo_sb = const.tile([128, 2 * 1024], BF16)
            bo_sb = const.tile([128, 8], F32)
            tex_sb = const.tile([128, 4 * TW], BF16)
            ident = const.tile([128, 128], BF16)
            QT = [const.tile([128, S], BF16, name=f"qt{h}") for h in range(4)]
            KT = [const.tile([128, S], BF16, name=f"kt{k}") for k in range(2)]
            vt_sb = const.tile([128, S], BF16)
            VS = [const.tile([128, 16 * 128], BF16, name=f"vs{k}") for k in range(2)]
            AT = [const.tile([128, S], BF16, name=f"at{c}") for c in range(2)]

            # ---- DMAs, priority order (sync queue is FIFO)
            nc.sync.dma_start(
                out=wk_sb.rearrange("p (c m) -> p c m", m=128),
                in_=wkT.rearrange("(c p) m -> p c m", p=128),
            )
            # x in 8 chunks of 2 e-blocks so the first projection matmuls
            # start ~2us in, with wq woven early for the Q blocks.
            for gi in range(8):
                qh, e0 = gi // 4, (gi % 4) * 2
                nc.sync.dma_start(
                    out=x_sb[
                        :, (qh * 8 + e0) * 1024 : (qh * 8 + e0 + 2) * 1024
                    ].rearrange("p (c m) -> p c m", m=1024),
                    in_=xT[e0 * 128 : (e0 + 2) * 128, qh * 1024 : (qh + 1) * 1024]
                    .rearrange("(c p) m -> p c m", p=128),
                )
                if gi == 1:
                    nc.sync.dma_start(
                        out=wq_sb.rearrange("p (c m) -> p c m", m=256),
                        in_=wqT.rearrange("(c p) m -> p c m", p=128),
                    )
            nc.sync.dma_start(
                out=wv_sb.rearrange("p (c m) -> p c m", m=128),
                in_=wvT.rearrange("(c p) m -> p c m", p=128),
            )
            nc.sync.dma_start(out=tex_sb[:, 0:TW], in_=texp[0])
            nc.sync.dma_start(out=bo_sb, in_=bo4[:, :])
            nc.sync.dma_start(
                out=wo_sb.rearrange("p (c m) -> p c m", m=1024),
                in_=woT.rearrange("(c p) m -> p c m", p=128),
            )
            for t in range(1, 4):
                nc.sync.dma_start(
                    out=tex_sb[:, t * TW : (t + 1) * TW], in_=texp[t]
                )

            # ---- one-time zeroing (S/PV stationaries + padded contraction
            # rows).  First-needed ones on DVE, the rest on idle gpsimd.
            nc.vector.memset(QT[0][64:128, :], 0.0)
            nc.vector.memset(KT[0][64:128, :], 0.0)
            nc.gpsimd.memset(KT[1][64:128, :], 0.0)
            for h in range(1, 4):
                nc.gpsimd.memset(QT[h][64:128, :], 0.0)
            for kv in range(2):
                nc.gpsimd.memset(VS[kv], 0.0)
                nc.gpsimd.memset(
                    VS[kv].rearrange("p (c m) -> p c m", m=128)[:, :, 64:65], 1.0
                )
            make_identity(nc, ident)

            def xc(qh, e):
                c = qh * 8 + e
                return x_sb[:, c * 1024 : (c + 1) * 1024]

            # ---- lead-in projections: 3-slot rotation (2x "s" + the idle
            # "pv" slot) so block i+2 never waits on block i's copies.
            def proj(wof, qh, copies, nm, tag="s"):
                bufs = 2 if tag == "s" else 1
                pst = psp.tile([128, 1024], F32, tag=tag, bufs=bufs, name=nm)
                for e in range(8):
                    x_ = xc(qh, e)
                    w = wof(e)
                    for i in range(2):
                        nc.tensor.matmul(
                            pst[:, i * 512 : (i + 1) * 512],
                            w,
                            x_[:, i * 512 : (i + 1) * 512],
                            start=(e == 0),
                            stop=(e == 7),
                        )
                copies(pst, qh)

            def k_copies(pst, qh):
                qs = slice(qh * 1024, (qh + 1) * 1024)
                nc.scalar.copy(KT[0][0:64, qs], pst[0:64, :])
                nc.scalar.copy(KT[1][0:64, qs], pst[64:128, :])

            def q01_copies(pst, qh):
                qs = slice(qh * 1024, (qh + 1) * 1024)
                nc.scalar.copy(QT[0][0:64, qs], pst[0:64, :])
                nc.scalar.copy(QT[1][0:64, qs], pst[64:128, :])

            def v_copies(pst, qh):
                qs = slice(qh * 1024, (qh + 1) * 1024)
                nc.vector.tensor_copy(vt_sb[:, qs], pst)

            # only K and Q(h0,h1) gate the S stream; V is projected inside
            # head 0 through the pv slot.
            wk_of = lambda e: wk_sb[:, e * 128 : (e + 1) * 128]
            wq_of = lambda e: wq_sb[:, e * 256 : e * 256 + 128]
            wv_of = lambda e: wv_sb[:, e * 128 : (e + 1) * 128]
            proj(wk_of, 0, k_copies, "pk", "s")
            proj(wq_of, 0, q01_copies, "pq", "s")
            proj(wq_of, 1, q01_copies, "pq", "pv")
            proj(wk_of, 1, k_copies, "pk", "s")

            # ---- weavable blocks (all live in the tag-"pv" slot, which is
            # idle until the first PV, so the S/exp stream never blocks)
            def big_transpose():
                # all 16 V^T->V transposes into ONE psum tile, then two big
                # strided copies into the VS stationaries (instead of 16
                # slot-serialized round-trips).
                bigpt = psp.tile([128, 2048], BF16, tag="pv", bufs=1, name="bigpt")
                for i in range(16):
                    nc.tensor.transpose(
                        bigpt[:, i * 128 : (i + 1) * 128],
                        vt_sb[:, i * 128 : (i + 1) * 128],
                        ident,
                    )
                for kv in range(2):
                    nc.vector.tensor_copy(
                        VS[kv].rearrange("p (c m) -> p c m", m=128)[:, :, 0:64],
                        bigpt.rearrange("p (c m) -> p c m", m=128)[
                            :, :, kv * 64 : (kv + 1) * 64
                        ],
                    )

            # half-block chunks through the pv slot, so each kc gets at most
            # ~8 extra matmuls and the ACT stream never starves.
            chunk_hold = {}

            def proj_chunk(key, wof, qh, part, copies):
                # half-block (8 matmul) chunks through the pv slot
                if part == 0:
                    chunk_hold[key] = psp.tile(
                        [128, 1024], F32, tag="pv", bufs=1, name=key
                    )
                pst = chunk_hold[key]
                for e in range(part * 4, part * 4 + 4):
                    x_ = xc(qh, e)
                    w = wof(e)
                    for i in range(2):
                        nc.tensor.matmul(
                            pst[:, i * 512 : (i + 1) * 512],
                            w,
                            x_[:, i * 512 : (i + 1) * 512],
                            start=(e == 0),
                            stop=(e == 7),
                        )
                if part == 1:
                    copies(pst, qh)

            wq23_of = lambda e: wq_sb[:, e * 256 + 128 : e * 256 + 256]

            def q23_copies(pst, qh):
                qs = slice(qh * 1024, (qh + 1) * 1024)
                nc.vector.tensor_copy(QT[2][0:64, qs], pst[0:64, :])
                nc.vector.tensor_copy(QT[3][0:64, qs], pst[64:128, :])

            # ---- attention stream
            def s_exp_mul(h, kc):
                kv = h // 2
                ptiles = []
                for q2 in range(2):
                    ss = psp.tile([128, 1024], F32, tag="s", bufs=2, name="ss")
                    for i in range(2):
                        qq = q2 * 2 + i
                        nc.tensor.matmul(
                            ss[:, i * 512 : (i + 1) * 512],
                            KT[kv][:, kc * 128 : (kc + 1) * 128],
                            QT[h][:, qq * 512 : (qq + 1) * 512],
                            start=True,
                            stop=True,
                        )
                    pexp = pp.tile([128, 1024], BF16, tag="pexp", bufs=3, name="pexp")
                    nc.scalar.activation(pexp, ss, Exp, scale=0.125)
                    ptile = pp.tile([128, 1024], BF16, tag="p", bufs=18, name="p")
                    u0 = h * TW + 1920 - kc * 128 + q2 * 1024
                    nc.vector.tensor_mul(ptile, pexp, tex_sb[:, u0 : u0 + 1024])
                    ptiles.append(ptile)
                return ptiles

            pvq = []
            pv_tiles = {}

            def norm_start(h):
                # Chain runs entirely off PE/PSUM-slots: DMA reshape, DVE
                # reciprocal, DMA back, gpsimd broadcast, DVE scale from PSUM.
                # Processed in pipelined q-halves to halve the latency until
                # the pv PSUM tile is released (it gates the next head's PV).
                # pv[0:64] is copied to SBUF immediately so the pv PSUM slot
                # frees ~4us after the last PV instead of after the whole
                # chain; the at-mul reads the SBUF copy.  This lets the next
                # head's first PV pop a few kc in with no catch-up lumps.
                pvt = pv_tiles[h]
                at = AT[h // 2]
                r0 = 64 * (h % 2)
                cs = [slice(half * 1024, (half + 1) * 1024) for half in range(2)]
                rsums, s128s, rrbs, rs, rbss, pvss = [], [], [], [], [], []
                for half in range(2):
                    rsum = nrm.tile([1, 1024], F32, tag="rsum", name="rsum")
                    nc.vector.tensor_copy(rsum, pvt[64:65, cs[half]])
                    rsums.append(rsum)
                    pvs = nrm.tile([64, 1024], BF16, tag="pvs", name="pvs")
                    nc.vector.tensor_copy(pvs, pvt[0:64, cs[half]])
                    pvss.append(pvs)
                for half in range(2):
                    s128 = nrm.tile([128, 8], F32, tag="s128", name="s128")
                    nc.sync.dma_start(out=s128, in_=rsums[half])
                    s128s.append(s128)
                for half in range(2):
                    rrb = nrm.tile([128, 8], BF16, tag="rrb", name="rrb")
                    with nc.allow_low_precision("1/rowsum rounds to bf16 anyway"):
                        nc.vector.reciprocal(rrb, s128s[half])
                    rrbs.append(rrb)
                for half in range(2):
                    r = nrm.tile([1, 1024], BF16, tag="r", name="r")
                    nc.sync.dma_start(out=r, in_=rrbs[half])
                    rs.append(r)
                for half in range(2):
                    rbs = nrm.tile([64, 1024], BF16, tag="rbs", name="rbs")
                    nc.gpsimd.partition_broadcast(rbs, rs[half])
                    rbss.append(rbs)
                # at-muls are pure-SBUF and latency-slack for h<3 (AT is only
                # read by phase D), so they run on the idle gpsimd engine;
                # h3's stay on DVE since they gate D's c1 matmuls.
                for half in range(2):
                    eng = nc.vector if h == 3 else nc.gpsimd
                    eng.tensor_mul(
                        at[r0 : r0 + 64, cs[half]], pvss[half], rbss[half]
                    )

            def pump(n):
                for _ in range(n):
                    if not pvq:
                        return
                    h, kc, pt_ = pvq.pop(0)
                    if h not in pv_tiles:
                        pv_tiles[h] = psp.tile(
                            [128, 2048], F32, tag="pv", bufs=1, name=f"pv{h}"
                        )
                    pvt = pv_tiles[h]
                    kv = h // 2
                    for qq in range(4):
                        nc.tensor.matmul(
                            pvt[:, qq * 512 : (qq + 1) * 512],
                            VS[kv][:, kc * 128 : (kc + 1) * 128],
                            pt_[qq // 2][:, (qq % 2) * 512 : (qq % 2 + 1) * 512],
                            start=(kc == 0),
                            stop=(kc == 15),
                            skip_group_check=True,
                        )
                    if kc == 15:
                        norm_start(h)

            # Schedule: each head's PVs drain within the head (lag ~6 kc,
            # gentle catch-up at kc12-15), so the normalization chain fires
            # right at head end and the next head's first PV (which waits on
            # it via the pv slot) only pops ~14us later.  The pv-tag slot is
            # FIFO: big_transpose and the q23 blocks must all precede the
            # first pump (which allocates pv(h0)).
            for h in range(4):
                for kc in range(16):
                    s_exp_mul_kc = s_exp_mul(h, kc)
                    pvq.append((h, kc, s_exp_mul_kc))
                    if h == 0:
                        # V projection 8mm/kc over kc0-3, transposes at kc5
                        if kc < 4:
                            proj_chunk("v0" if kc < 2 else "v1", wv_of, kc // 2,
                                       kc % 2, v_copies)
                        elif kc == 5:
                            big_transpose()
                        if 7 <= kc <= 11:
                            pump(1)
                        elif 12 <= kc <= 13:
                            pump(2)
                        elif kc >= 14:
                            pump(3 if kc == 14 else 4)
                    elif h == 1:
                        # Q(h2,h3) projections 8mm/kc at kc2-5 (pv0 frees
                        # right at the boundary now, so kc2 is safe).
                        if 2 <= kc <= 5:
                            proj_chunk("q23a" if kc < 4 else "q23b", wq23_of,
                                       (kc - 2) // 2, kc % 2, q23_copies)
                        if 6 <= kc <= 11:
                            pump(1)
                        elif 12 <= kc <= 15:
                            pump(2 if kc < 14 else 3)
                    else:
                        if 4 <= kc <= 11:
                            pump(1)
                        elif 12 <= kc <= 15:
                            pump(2)
            pump(len(pvq))

            # ---- output projection (+ bias/4 fused into the PSUM drain)
            # Even ec use the (now free) 4-bank "pv" slot, odd ec the two
            # "s" slots, so three tiles pipeline through PSUM.
            def d_drain(os_, ec, half):
                o_sb = osb.tile([128, 1024], BF16, tag="osb", name="o_sb")
                nc.vector.tensor_scalar_add(o_sb, os_, bo_sb[:, ec : ec + 1])
                nc.sync.dma_start(
                    out=outT[
                        ec * 128 : (ec + 1) * 128,
                        half * 1024 : (half + 1) * 1024,
                    ],
                    in_=o_sb,
                )

            # half 0 first: its c1 matmuls only need the h3 at-mul for
            # columns 0:1023, which lands ~2us before the half-1 one.  ec6/7
            # ride the freed pv slot between the passes for a 3rd buffer.
            # 4 ecs drain through the pv slot as (ec, full 2048): one wide
            # activation + one contiguous DMA each; the other 4 ecs go as
            # (ec, half) tiles through the two "s" slots.  All c0 matmuls of
            # a group precede its c1s so they overlap the h3 norm chain.
            def d_group(ec_s, ec_w, half_tiles_first):
                tiles = []
                for ec, half in half_tiles_first:
                    os_ = psp.tile([128, 1024], F32, tag="s", bufs=2, name="os")
                    tiles.append((os_, 0, [(ec, 0, half * 2), (ec, 512, half * 2 + 1)]))
                os2 = psp.tile([128, 2048], F32, tag="pv", bufs=1, name="os2")
                tiles.append(
                    (os2, 1, [(ec_w, qq * 512, qq) for qq in range(4)])
                )
                for c in range(2):
                    for os_, _, mms in tiles:
                        for ec, off, qq in mms:
                            w = wo_sb[
                                :, c * 1024 + ec * 128 : c * 1024 + (ec + 1) * 128
                            ]
                            nc.tensor.matmul(
                                os_[:, off : off + 512],
                                w,
                                AT[c][:, qq * 512 : (qq + 1) * 512],
                                start=(c == 0),
                                stop=(c == 1),
                                skip_group_check=True,
                            )
                for os_, wide, mms in tiles:
                    if wide:
                        ec = mms[0][0]
                        o_sb = osb.tile(
                            [128, 2048], BF16, tag="osbw", bufs=2, name="o_sb"
                        )
                        nc.scalar.activation(
                            o_sb, os_, Ident, bias=bo_sb[:, ec : ec + 1]
                        )
                        nc.sync.dma_start(
                            out=outT[ec * 128 : (ec + 1) * 128, :], in_=o_sb
                        )
                    else:
                        d_drain(os_, mms[0][0], mms[0][2] // 2)

            # ec 1,3,5,7 via wide pv tiles; ec 0,2,4,6 via per-half s tiles
            d_group(None, 1, [(0, 0), (2, 0)])
            d_group(None, 3, [(4, 0), (6, 0)])
            d_group(None, 5, [(0, 1), (2, 1)])
            d_group(None, 7, [(4, 1), (6, 1)])

    nc.compile()
    return nc


def _texp_tables():
    i = np.arange(128, dtype=np.float64).reshape(128, 1)
    u = np.arange(TW, dtype=np.float64).reshape(1, TW)
    dist = np.abs(i + 1920.0 - u)
    tabs = []
    for g in range(4):
        tg = np.empty([4, 128, TW], dtype=ml_dtypes.bfloat16)
        for hh in range(4):
            slope = 2.0 ** (-(4 * g + hh + 1))
            tg[hh] = np.exp(-slope * dist / 8.0).astype(ml_dtypes.bfloat16)
        tabs.append(tg)
    return tabs


def kernel(x, Wq, Wk, Wv, Wo, bo, _trace=False, _trace_kwargs=None):
    global _NC, LAST_RESULTS
    x = np.asarray(x, dtype=np.float32)
    Wq = np.asarray(Wq, dtype=np.float32)
    Wk = np.asarray(Wk, dtype=np.float32)
    Wv = np.asarray(Wv, dtype=np.float32)
    Wo = np.asarray(Wo, dtype=np.float32)
    bo = np.asarray(bo, dtype=np.float32)

    if _NC is None:
        _NC = _build()
    nc = _NC

    tabs = _texp_tables()
    bf = ml_dtypes.bfloat16
    bo4 = np.ascontiguousarray((bo * 0.25).reshape(8, 128).T).astype(np.float32)
    in_maps = []
    for core in range(8):
        n, g = core // 4, core % 4
        hs = slice(4 * g * D, (4 * g + 4) * D)
        kvs = slice(2 * g * D, (2 * g + 2) * D)
        in_maps.append(
            {
                "xT": np.ascontiguousarray(x[n].T).astype(bf),
                "wqT": np.ascontiguousarray(Wq[hs].T).astype(bf),
                "wkT": np.ascontiguousarray(Wk[kvs].T).astype(bf),
                "wvT": np.ascontiguousarray(Wv[kvs].T).astype(bf),
                "woT": np.ascontiguousarray(Wo[:, hs].T).astype(bf),
                "bo4": bo4,
                "texp": tabs[g],
            }
        )

    kw = {}
    if _trace:
        kw["trace"] = True
        kw.update(_trace_kwargs or {})
    res = run_bass_kernel_spmd(nc, in_maps, list(range(8)), **kw)
    LAST_RESULTS = res

    out = np.empty((2, S, E), dtype=np.float32)
    for n in range(2):
        acc = res.results[n * 4]["outT"].astype(np.float32)
        for g in range(1, 4):
            acc = acc + res.results[n * 4 + g]["outT"]
        out[n] = acc.T
    return out
